# revision 1
# baseline (speedup 1.0000x reference)
"""KGE (TransR-style) loss kernel for Trainium2, 8 NeuronCores.

Strategy:
  - Host: sort the M=8192 triples by relation id (pure index manipulation),
    pad each relation's segment to 128-row blocks -> ~96 single-relation
    blocks, distributed evenly across the 8 cores (same block count per
    core, so one SPMD program serves all cores). Per-core relation tables
    (W blocks, r rows) are sharded host-side per the block list.
  - Device (per core, per block b):
      * three indirect DMAs gather the h/pos/neg entity rows into
        X = [H | P | N]  (128 x 384)   [GPSIMD/SWDGE]
      * D_pos = H - P, D_neg = H - N; squares + row reductions  [DVE]
      * PE transpose D -> D^T; ACT copies PSUM->SBUF
      * matmul D^T.T @ W_b accumulated with a K=NB one-hot matmul adding
        r_b -> (h - t) @ W + r in PSUM  [PE]
      * score diff col stored per block; softplus tail batched over all
        blocks at the end (2 act-table loads total instead of ~2/block)
  - reg = 0.5*sum(X^2) per row, masked+scaled by 1e-5 via the wval input;
    relation-embedding reg via per-block counts.
  - Final: free-dim reduce + ones-matmul partition reduce -> one f32 per
    core; host sums the 8 partials and divides by M.
"""

import os
from contextlib import ExitStack

import numpy as np

import concourse.bass as bass
import concourse.tile as tile
from concourse import bacc, mybir
from concourse.masks import make_identity

M = 8192
E = 128
N_ENT = 500000
N_REL = 64
LAM = 1e-5
P = 128
N_CORES = 8
PAD_BIAS = -30000.0

f32 = mybir.dt.float32
i32 = mybir.dt.int32

_cache = {}


def _build(NB: int):
    """Build + compile the single-core SPMD program for NB blocks/core."""
    nc = bacc.Bacc(
        "TRN2",
        target_bir_lowering=False,
        debug=False,
        num_devices=N_CORES,
    )

    ent = nc.dram_tensor("ent", (N_ENT, E), f32, kind="ExternalInput").ap()
    idx3 = nc.dram_tensor("idx3", (P, NB * 3), i32, kind="ExternalInput").ap()
    mbias = nc.dram_tensor("mbias", (P, NB), f32, kind="ExternalInput").ap()
    wval = nc.dram_tensor("wval", (P, NB), f32, kind="ExternalInput").ap()
    w_all = nc.dram_tensor("w_all", (P, NB * P), f32, kind="ExternalInput").ap()
    r_in = nc.dram_tensor("r_blk", (NB, E), f32, kind="ExternalInput").ap()
    lsel = nc.dram_tensor("lsel", (NB, NB * P), f32, kind="ExternalInput").ap()
    cnt = nc.dram_tensor("cnt", (NB, 1), f32, kind="ExternalInput").ap()
    out = nc.dram_tensor("out", (1, 1), f32, kind="ExternalOutput").ap()

    with tile.TileContext(nc) as tc, ExitStack() as ctx:
        const = ctx.enter_context(tc.tile_pool(name="const", bufs=1))
        xp = ctx.enter_context(tc.tile_pool(name="xp", bufs=6))
        dp = ctx.enter_context(tc.tile_pool(name="dp", bufs=3))
        dtp = ctx.enter_context(tc.tile_pool(name="dtp", bufs=3))
        scrp = ctx.enter_context(tc.tile_pool(name="scrp", bufs=3))
        colp = ctx.enter_context(tc.tile_pool(name="colp", bufs=4))
        ps_t = ctx.enter_context(tc.tile_pool(name="ps_t", bufs=2, space="PSUM"))
        ps_mm = ctx.enter_context(tc.tile_pool(name="ps_mm", bufs=2, space="PSUM"))

        # constants / small inputs
        iden = const.tile([P, P], f32)
        make_identity(nc, iden[:])
        ones_col = const.tile([P, 1], f32)
        nc.gpsimd.memset(ones_col[:], 1.0)

        idx3_sb = const.tile([P, NB * 3], i32)
        nc.sync.dma_start(out=idx3_sb[:], in_=idx3[:])
        mb_sb = const.tile([P, NB], f32)
        nc.sync.dma_start(out=mb_sb[:], in_=mbias[:])
        wv_sb = const.tile([P, NB], f32)
        nc.sync.dma_start(out=wv_sb[:], in_=wval[:])
        cnt_sb = const.tile([NB, 1], f32)
        nc.sync.dma_start(out=cnt_sb[:], in_=cnt[:])
        w_sb = const.tile([P, NB * P], f32)
        nc.sync.dma_start(out=w_sb[:], in_=w_all[:])
        r_blk = const.tile([NB, E], f32)
        nc.sync.dma_start(out=r_blk[:], in_=r_in[:])
        lsel_sb = const.tile([NB, NB * P], f32)
        nc.sync.dma_start(out=lsel_sb[:], in_=lsel[:])

        # per-block score-diff columns and raw reg columns
        dcols = const.tile([P, NB], f32)
        regs = const.tile([P, NB], f32)

        for b in range(NB):
            # three gathers: hardware indirect DMA takes one index per
            # partition and reads out.free_size contiguous elems from it
            x = xp.tile([P, 3 * E], f32, tag="x")
            for j in range(3):
                nc.gpsimd.indirect_dma_start(
                    out=x[:, j * E : (j + 1) * E],
                    out_offset=None,
                    in_=ent[:],
                    in_offset=bass.IndirectOffsetOnAxis(
                        ap=idx3_sb[:, 3 * b + j : 3 * b + j + 1], axis=0
                    ),
                )

            # raw reg col: sum over [H|P|N] of squares (mask+scale at tail);
            # ACT Square with accum_out frees the DVE for score work
            xsq = scrp.tile([P, 3 * E], f32, tag="xsq")
            nc.scalar.activation(
                out=xsq[:], in_=x[:],
                func=mybir.ActivationFunctionType.Square,
                accum_out=regs[:, b : b + 1],
            )

            # D_pos = H - P, D_neg = H - N
            d_pos = dp.tile([P, E], f32, tag="dpos")
            nc.vector.tensor_tensor(
                out=d_pos[:], in0=x[:, 0:E], in1=x[:, E : 2 * E],
                op=mybir.AluOpType.subtract,
            )
            d_neg = dp.tile([P, E], f32, tag="dneg")
            nc.vector.tensor_tensor(
                out=d_neg[:], in0=x[:, 0:E], in1=x[:, 2 * E : 3 * E],
                op=mybir.AluOpType.subtract,
            )

            # transpose D -> D^T (PSUM), copy to SBUF on ACT
            dpt_ps = ps_t.tile([P, P], f32, tag="tp")
            nc.tensor.transpose(out=dpt_ps[:], in_=d_pos[:], identity=iden[:])
            dnt_ps = ps_t.tile([P, P], f32, tag="tn")
            nc.tensor.transpose(out=dnt_ps[:], in_=d_neg[:], identity=iden[:])
            dpt = dtp.tile([P, P], f32, tag="dpt")
            nc.scalar.copy(dpt[:], dpt_ps[:])
            dnt = dtp.tile([P, P], f32, tag="dnt")
            nc.scalar.copy(dnt[:], dnt_ps[:])

            # (h - t) @ W + r
            wb = w_sb[:, b * P : (b + 1) * P]
            lb = lsel_sb[:, b * P : (b + 1) * P]
            pos_ps = ps_mm.tile([P, E], f32, tag="mp")
            nc.tensor.matmul(out=pos_ps[:], lhsT=dpt[:], rhs=wb, start=True, stop=False)
            nc.tensor.matmul(out=pos_ps[:], lhsT=lb, rhs=r_blk[:], start=False, stop=True)
            neg_ps = ps_mm.tile([P, E], f32, tag="mn")
            nc.tensor.matmul(out=neg_ps[:], lhsT=dnt[:], rhs=wb, start=True, stop=False)
            nc.tensor.matmul(out=neg_ps[:], lhsT=lb, rhs=r_blk[:], start=False, stop=True)

            # score diff col (x2): sum(neg^2) - sum(pos^2); ACT Square reads
            # PSUM (DVE cannot read two PSUM inputs) and fuses the reduction
            psq = scrp.tile([P, E], f32, tag="psq")
            spos = colp.tile([P, 1], f32, tag="sp")
            nc.scalar.activation(
                out=psq[:], in_=pos_ps[:],
                func=mybir.ActivationFunctionType.Square,
                accum_out=spos[:],
            )
            nsq = scrp.tile([P, E], f32, tag="nsq")
            sneg = colp.tile([P, 1], f32, tag="sn")
            nc.scalar.activation(
                out=nsq[:], in_=neg_ps[:],
                func=mybir.ActivationFunctionType.Square,
                accum_out=sneg[:],
            )
            nc.vector.tensor_tensor(
                out=dcols[:, b : b + 1], in0=sneg[:], in1=spos[:],
                op=mybir.AluOpType.subtract,
            )

        # ---- batched tail over all NB blocks ----
        # loss = softplus(0.5*dcols + mbias) = relu(y) + ln(1 + exp(-|y|))
        dm = const.tile([P, NB], f32)
        nc.vector.tensor_scalar_mul(out=dm[:], in0=dcols[:], scalar1=0.5)
        nc.vector.tensor_tensor(
            out=dm[:], in0=dm[:], in1=mb_sb[:], op=mybir.AluOpType.add
        )
        t_abs = const.tile([P, NB], f32)
        nc.scalar.activation(
            out=t_abs[:], in_=dm[:], func=mybir.ActivationFunctionType.Abs
        )
        t_exp = const.tile([P, NB], f32)
        nc.scalar.activation(
            out=t_exp[:], in_=t_abs[:], func=mybir.ActivationFunctionType.Exp,
            scale=-1.0,
        )
        t_ln = const.tile([P, NB], f32)
        nc.scalar.activation(
            out=t_ln[:], in_=t_exp[:], func=mybir.ActivationFunctionType.Ln,
            bias=1.0,
        )
        t_relu = const.tile([P, NB], f32)
        nc.scalar.activation(
            out=t_relu[:], in_=dm[:], func=mybir.ActivationFunctionType.Relu
        )

        acc = const.tile([P, 2 * NB], f32)
        nc.vector.tensor_tensor(
            out=acc[:, :NB], in0=t_ln[:], in1=t_relu[:], op=mybir.AluOpType.add
        )
        # reg masked + scaled (wval holds 0.5*1e-5 or 0)
        nc.vector.tensor_tensor(
            out=acc[:, NB:], in0=regs[:], in1=wv_sb[:], op=mybir.AluOpType.mult
        )

        # relation-embedding reg: cnt_b * 0.5*||r_b||^2 (cnt pre-scaled 1e-5)
        rsq = const.tile([NB, E], f32)
        nc.vector.tensor_tensor(
            out=rsq[:], in0=r_blk[:], in1=r_blk[:], op=mybir.AluOpType.mult
        )
        rr_col = const.tile([NB, 1], f32)
        nc.vector.reduce_sum(out=rr_col[:], in_=rsq[:], axis=mybir.AxisListType.X)
        rr_s = const.tile([NB, 1], f32)
        nc.vector.tensor_tensor(
            out=rr_s[:], in0=rr_col[:], in1=cnt_sb[:], op=mybir.AluOpType.mult
        )

        # total per-partition, then partition-reduce via ones matmul
        t_all = const.tile([P, 1], f32)
        nc.vector.reduce_sum(out=t_all[:], in_=acc[:], axis=mybir.AxisListType.X)
        nc.vector.tensor_tensor(
            out=t_all[:NB], in0=t_all[:NB], in1=rr_s[:], op=mybir.AluOpType.add
        )
        fin_ps = ps_mm.tile([1, 1], f32, tag="mp")
        nc.tensor.matmul(out=fin_ps[:], lhsT=t_all[:], rhs=ones_col[:], start=True, stop=True)
        fin_sb = const.tile([1, 1], f32)
        nc.scalar.copy(fin_sb[:], fin_ps[:])
        nc.sync.dma_start(out=out[:], in_=fin_sb[:])

    nc.compile()
    return nc


def _plan(h, r, pos_t, neg_t, relation_weight, relation_embed):
    """Sort by relation, pad to 128-row single-relation blocks, split 8 ways."""
    order = np.argsort(r, kind="stable")
    counts = np.bincount(r, minlength=N_REL)
    blocks = []
    pos = 0
    for k in range(N_REL):
        c = int(counts[k])
        ids = order[pos : pos + c]
        pos += c
        for s in range(0, c, P):
            blocks.append((k, ids[s : s + P]))
    nb = max(2, -(-len(blocks) // N_CORES))
    while len(blocks) < nb * N_CORES:
        blocks.append((0, np.empty(0, np.int64)))

    maps = []
    for c in range(N_CORES):
        core_blocks = blocks[c * nb : (c + 1) * nb]
        idx3 = np.zeros((P, nb, 3), np.int32)
        mb = np.full((P, nb), PAD_BIAS, np.float32)
        wv = np.zeros((P, nb), np.float32)
        cnt = np.zeros((nb, 1), np.float32)
        w_blk = np.zeros((P, nb, P), np.float32)
        r_blk = np.zeros((nb, E), np.float32)
        for b, (k, ids) in enumerate(core_blocks):
            n = len(ids)
            if n:
                idx3[:n, b, 0] = h[ids]
                idx3[:n, b, 1] = pos_t[ids]
                idx3[:n, b, 2] = neg_t[ids]
            mb[:n, b] = 0.0
            wv[:n, b] = 0.5 * LAM
            cnt[b, 0] = n * LAM
            w_blk[:, b, :] = relation_weight[k]
            r_blk[b, :] = relation_embed[k]
        maps.append(
            {
                "idx3": idx3.reshape(P, nb * 3),
                "mbias": mb,
                "wval": wv,
                "cnt": cnt,
                "w_all": np.ascontiguousarray(w_blk.reshape(P, nb * P)),
                "r_blk": r_blk,
                "lsel": np.kron(np.eye(nb, dtype=np.float32), np.ones((1, P), np.float32)),
            }
        )
    return nb, maps


def kernel(h, r, pos_t, neg_t, entity_embed, relation_embed, relation_weight):
    h = np.asarray(h).astype(np.int32)
    r = np.asarray(r).astype(np.int32)
    pos_t = np.asarray(pos_t).astype(np.int32)
    neg_t = np.asarray(neg_t).astype(np.int32)
    ent = np.ascontiguousarray(np.asarray(entity_embed, dtype=np.float32))
    re = np.ascontiguousarray(np.asarray(relation_embed, dtype=np.float32))
    rw = np.ascontiguousarray(np.asarray(relation_weight, dtype=np.float32))

    nb, maps = _plan(h, r, pos_t, neg_t, rw, re)
    if nb not in _cache:
        _cache[nb] = _build(nb)
    nc = _cache[nb]

    in_maps = [{"ent": ent, **maps[c]} for c in range(N_CORES)]

    if os.environ.get("KGE_SIM"):
        from concourse.bass_interp import CoreSim

        total = 0.0
        for c in range(N_CORES):
            sim = CoreSim(nc, trace=False)
            for name, arr in in_maps[c].items():
                sim.tensor(name)[:] = arr
            sim.simulate()
            total += float(sim.tensor("out")[0, 0])
        return np.float32(total / M)

    from concourse.bass_utils import run_bass_kernel_spmd

    res = run_bass_kernel_spmd(nc, in_maps, core_ids=list(range(N_CORES)))
    total = sum(float(res.results[c]["out"][0, 0]) for c in range(N_CORES))
    return np.float32(total / M)



# revision 4
# speedup vs baseline: 1.6607x; 1.6607x over previous
"""KGE (TransR-style) loss kernel for Trainium2, 8 NeuronCores.

Strategy (v2):
  - Host: sort the M=8192 triples by relation id, pad each relation's
    segment to 64-row runs -> ~80 blocks of 128 rows where every block is
    exactly two single-relation 64-row runs.  10 blocks/core, one SPMD
    program for all 8 cores.  W tables are packed one slot per run
    (duplicated as needed) so all device-side APs are static.
  - Device (per core):
      * ONE indirect DMA per half gathers all h/pos/neg entity rows into
        X = [H|P|N] per block  (2 SWDGE instructions instead of 36: the
        994ns/instr descriptor-generation overhead dominated the old
        kernel)
      * D_pos = H - P, D_neg = H - N on DVE as two wide strided
        instructions, f32 in -> bf16 out
      * PE transposes D -> D^T in bf16 (1 cyc/row), 8 tiles per PSUM
        bank, one wide ACT copy per bank to SBUF
      * per block: V = [D_pos@W | D_neg@W] + r in one PSUM bank via
        5 bf16 matmuls (one K=16 one-hot matmul adds r to both halves,
        then 4 64-row run matmuls with the run's W slot)
      * ACT Square V -> bf16 squares in SBUF (one instr per block)
      * ONE segmented DVE reduce (axis=X over [128, 2NB, 128]) gives all
        scores; softplus tail batched over [128, NB]
  - Final: free-dim reduce + ones-matmul partition reduce -> one f32 per
    core; host sums the 8 partials, divides by M and adds the (tiny,
    1e-5-scaled) embedding-norm regularizer computed on host in fp64.
"""

import os
from contextlib import ExitStack

import numpy as np

import concourse.bass as bass
import concourse.tile as tile
from concourse import bacc, mybir

M = 8192
E = 128
N_ENT = 500000
N_REL = 64
LAM = 1e-5
P = 128
RUN = 64
KSEL = 16
N_CORES = 8
PAD_BIAS = -30000.0

f32 = mybir.dt.float32
bf16 = mybir.dt.bfloat16
i32 = mybir.dt.int32

_cache = {}


def _build(NB: int):
    """Build + compile the single-core SPMD program for NB blocks/core."""
    nc = bacc.Bacc(
        "TRN2",
        target_bir_lowering=False,
        debug=False,
        num_devices=N_CORES,
    )

    ent = nc.dram_tensor("ent", (N_ENT, E), f32, kind="ExternalInput").ap()
    idx3 = nc.dram_tensor("idx3", (P, NB * 3), i32, kind="ExternalInput").ap()
    # mbias columns + ones column for the final partition reduce
    mbias = nc.dram_tensor("mbias", (P, NB + 1), f32, kind="ExternalInput").ap()
    wsl = nc.dram_tensor("wsl", (P, 2 * NB * P), bf16, kind="ExternalInput").ap()
    # sel one-hot [KSEL, NB*P] followed by r2 = [r|r] [KSEL, 2P]
    selr = nc.dram_tensor("selr", (KSEL, NB * P + 2 * P), bf16, kind="ExternalInput").ap()
    idn = nc.dram_tensor("idn", (P, P), bf16, kind="ExternalInput").ap()
    out = nc.dram_tensor("out", (1, 1), f32, kind="ExternalOutput").ap()

    c0 = NB - NB // 2  # blocks in chunk 0
    chunks = [(0, c0), (c0, NB - c0)] if NB > c0 else [(0, c0)]

    with tile.TileContext(nc) as tc, ExitStack() as ctx:
        const = ctx.enter_context(tc.tile_pool(name="const", bufs=1))
        dtp = ctx.enter_context(tc.tile_pool(name="dtp", bufs=2, space="PSUM"))
        vp = ctx.enter_context(tc.tile_pool(name="vp", bufs=4, space="PSUM"))
        fp = ctx.enter_context(tc.tile_pool(name="fp", bufs=1, space="PSUM"))

        # ---- input loads ----
        idx_sb = const.tile([P, 3 * NB], i32)
        nc.sync.dma_start(out=idx_sb[:], in_=idx3[:])
        w_sb = const.tile([P, 2 * NB * P], bf16)
        nc.sync.dma_start(out=w_sb[:], in_=wsl[:])
        selr_sb = const.tile([KSEL, NB * P + 2 * P], bf16)
        nc.sync.dma_start(out=selr_sb[:], in_=selr[:])
        mb_sb = const.tile([P, NB + 1], f32)
        nc.sync.dma_start(out=mb_sb[:], in_=mbias[:])
        idn_sb = const.tile([P, P], bf16)
        nc.sync.dma_start(out=idn_sb[:], in_=idn[:])

        xs = []
        for g, (b0, cg) in enumerate(chunks):
            x = const.tile([P, cg * 3 * E], f32)
            nc.gpsimd.indirect_dma_start(
                out=x[:],
                out_offset=None,
                in_=ent[:],
                in_offset=bass.IndirectOffsetOnAxis(
                    ap=idx_sb[:, 3 * b0 : 3 * (b0 + cg)], axis=0
                ),
            )
            xs.append(x)

        # D_pos/D_neg per chunk: wide strided subtract, f32 -> bf16
        dps, dns = [], []
        for g, (b0, cg) in enumerate(chunks):
            x3 = xs[g][:].rearrange("p (b x) -> p b x", x=3 * E)
            dp = const.tile([P, cg * E], bf16)
            nc.vector.tensor_tensor(
                out=dp[:].rearrange("p (b x) -> p b x", x=E),
                in0=x3[:, :, 0:E],
                in1=x3[:, :, E : 2 * E],
                op=mybir.AluOpType.subtract,
            )
            dn = const.tile([P, cg * E], bf16)
            nc.vector.tensor_tensor(
                out=dn[:].rearrange("p (b x) -> p b x", x=E),
                in0=x3[:, :, 0:E],
                in1=x3[:, :, 2 * E : 3 * E],
                op=mybir.AluOpType.subtract,
            )
            dps.append(dp)
            dns.append(dn)

        # transposes: per chunk, groups of up to 4 blocks -> one PSUM bank
        # (8 bf16 [128,128] tiles), one wide ACT copy to SBUF per bank
        dT = const.tile([P, 2 * NB * E], bf16)  # block b: pos at 2b, neg at 2b+1
        sq_all = const.tile([P, 2 * NB * E], bf16)
        scores = const.tile([P, 2 * NB], f32)

        r2 = selr_sb[:, NB * P : NB * P + 2 * P]

        for g, (b0, cg) in enumerate(chunks):
            for g4 in range(0, cg, 4):
                n4 = min(4, cg - g4)
                dt_ps = dtp.tile([P, 8 * E], bf16, tag="dt")
                for i in range(n4):
                    bl = g4 + i  # block local to chunk
                    nc.tensor.transpose(
                        out=dt_ps[:, (2 * i) * E : (2 * i + 1) * E],
                        in_=dps[g][:, bl * E : (bl + 1) * E],
                        identity=idn_sb[:],
                    )
                    nc.tensor.transpose(
                        out=dt_ps[:, (2 * i + 1) * E : (2 * i + 2) * E],
                        in_=dns[g][:, bl * E : (bl + 1) * E],
                        identity=idn_sb[:],
                    )
                dst0 = 2 * (b0 + g4) * E
                nc.scalar.copy(
                    dT[:, dst0 : dst0 + 2 * n4 * E], dt_ps[:, 0 : 2 * n4 * E]
                )

            # matmuls + squares for this chunk's blocks
            for bl in range(cg):
                b = b0 + bl
                v_ps = vp.tile([P, 512], f32, tag="v")
                # r add: one K=16 one-hot matmul covers pos and neg halves
                nc.tensor.matmul(
                    out=v_ps[:, 0:256],
                    lhsT=selr_sb[:, b * P : (b + 1) * P],
                    rhs=r2,
                    start=True,
                    stop=False,
                )
                dposT = dT[:, (2 * b) * E : (2 * b + 1) * E]
                dnegT = dT[:, (2 * b + 1) * E : (2 * b + 2) * E]
                for u in range(2):
                    wslot = w_sb[:, (2 * b + u) * P : (2 * b + u + 1) * P]
                    rows = slice(u * RUN, (u + 1) * RUN)
                    nc.tensor.matmul(
                        out=v_ps[rows, 0:128],
                        lhsT=dposT[:, rows],
                        rhs=wslot,
                        start=False,
                        stop=False,
                    )
                    nc.tensor.matmul(
                        out=v_ps[rows, 128:256],
                        lhsT=dnegT[:, rows],
                        rhs=wslot,
                        start=False,
                        stop=True,
                    )
                nc.scalar.activation(
                    out=sq_all[:, b * 256 : (b + 1) * 256],
                    in_=v_ps[:, 0:256],
                    func=mybir.ActivationFunctionType.Square,
                )

        # ---- batched tail ----
        # scores[p, 2b] = sum pos sq, [p, 2b+1] = sum neg sq
        nc.vector.reduce_sum(
            out=scores[:],
            in_=sq_all[:].rearrange("p (s x) -> p s x", x=E),
            axis=mybir.AxisListType.X,
        )
        s2 = scores[:].rearrange("p (b two) -> p b two", two=2)
        dcol = const.tile([P, NB], f32)
        nc.vector.tensor_tensor(
            out=dcol[:].unsqueeze(2),
            in0=s2[:, :, 1:2],
            in1=s2[:, :, 0:1],
            op=mybir.AluOpType.subtract,
        )
        y = const.tile([P, NB], f32)
        nc.vector.tensor_scalar_mul(out=y[:], in0=dcol[:], scalar1=0.5)
        nc.vector.tensor_tensor(
            out=y[:], in0=y[:], in1=mb_sb[:, 0:NB], op=mybir.AluOpType.add
        )
        # softplus(y) = relu(y) + ln(1 + exp(-|y|))
        t_abs = const.tile([P, NB], f32)
        nc.scalar.activation(
            out=t_abs[:], in_=y[:], func=mybir.ActivationFunctionType.Abs
        )
        t_exp = const.tile([P, NB], f32)
        nc.scalar.activation(
            out=t_exp[:], in_=t_abs[:], func=mybir.ActivationFunctionType.Exp,
            scale=-1.0,
        )
        t_ln = const.tile([P, NB], f32)
        nc.scalar.activation(
            out=t_ln[:], in_=t_exp[:], func=mybir.ActivationFunctionType.Ln,
            bias=1.0,
        )
        t_relu = const.tile([P, NB], f32)
        nc.scalar.activation(
            out=t_relu[:], in_=y[:], func=mybir.ActivationFunctionType.Relu
        )
        acc = const.tile([P, NB], f32)
        nc.vector.tensor_tensor(
            out=acc[:], in0=t_ln[:], in1=t_relu[:], op=mybir.AluOpType.add
        )
        tcol = const.tile([P, 1], f32)
        nc.vector.reduce_sum(out=tcol[:], in_=acc[:], axis=mybir.AxisListType.X)
        fin_ps = fp.tile([1, 1], f32, tag="fin")
        nc.tensor.matmul(
            out=fin_ps[:], lhsT=tcol[:], rhs=mb_sb[:, NB : NB + 1],
            start=True, stop=True,
        )
        fin_sb = const.tile([1, 1], f32)
        nc.scalar.copy(fin_sb[:], fin_ps[:])
        nc.sync.dma_start(out=out[:], in_=fin_sb[:])

    nc.compile()
    return nc


def _plan(h, r, pos_t, neg_t, relation_weight, relation_embed):
    """Sort by relation, pad to 64-row single-relation runs, split 8 ways."""
    order = np.argsort(r, kind="stable")
    counts = np.bincount(r, minlength=N_REL)
    rows_parts, rel_parts = [], []
    pos = 0
    for k in range(N_REL):
        c = int(counts[k])
        ids = order[pos : pos + c]
        pos += c
        if c == 0:
            continue
        n_slots = -(-c // RUN) * RUN
        arr = np.full(n_slots, -1, np.int64)
        arr[:c] = ids
        rows_parts.append(arr)
        rel_parts.append(np.full(n_slots, k, np.int64))
    rows = np.concatenate(rows_parts)
    rels = np.concatenate(rel_parts)
    t0 = len(rows)
    nb = max(1, -(-t0 // (P * N_CORES)))
    t = nb * P * N_CORES
    if t > t0:
        rows = np.concatenate([rows, np.full(t - t0, -1, np.int64)])
        rels = np.concatenate([rels, np.zeros(t - t0, np.int64)])

    iden = np.eye(P, dtype=np.float32)

    maps = []
    for c in range(N_CORES):
        sl_rows = rows[c * nb * P : (c + 1) * nb * P]
        sl_rels = rels[c * nb * P : (c + 1) * nb * P]
        core_rels = []
        for k in sl_rels[::RUN]:
            if k not in core_rels:
                core_rels.append(int(k))
        assert len(core_rels) <= KSEL, f"core {c} has {len(core_rels)} relations"
        rel_slot = {k: i for i, k in enumerate(core_rels)}

        idx3 = np.zeros((P, nb, 3), np.int32)
        mb = np.full((P, nb + 1), PAD_BIAS, np.float32)
        mb[:, nb] = 1.0  # ones column for the final reduce
        wv = np.zeros((P, 2 * nb, P), np.float32)
        sel = np.zeros((KSEL, nb * P + 2 * P), np.float32)
        for b in range(nb):
            for u in range(2):
                k = int(sl_rels[(2 * b + u) * RUN])
                wv[:, 2 * b + u, :] = relation_weight[k]
            for p in range(P):
                s = b * P + p
                rid = sl_rows[s]
                if rid >= 0:
                    idx3[p, b, 0] = h[rid]
                    idx3[p, b, 1] = pos_t[rid]
                    idx3[p, b, 2] = neg_t[rid]
                    mb[p, b] = 0.0
                    sel[rel_slot[int(sl_rels[s])], s] = 1.0
        for i, k in enumerate(core_rels):
            sel[i, nb * P : nb * P + P] = relation_embed[k]
            sel[i, nb * P + P : nb * P + 2 * P] = relation_embed[k]
        maps.append(
            {
                "idx3": idx3.reshape(P, nb * 3),
                "mbias": mb,
                "wsl": wv.reshape(P, 2 * nb * P),
                "selr": sel,
                "idn": iden,
            }
        )
    return nb, maps


def _to_bf16(a):
    import ml_dtypes

    return np.asarray(a, dtype=np.float32).astype(ml_dtypes.bfloat16)


def kernel(h, r, pos_t, neg_t, entity_embed, relation_embed, relation_weight):
    h = np.asarray(h).astype(np.int64)
    r = np.asarray(r).astype(np.int64)
    pos_t = np.asarray(pos_t).astype(np.int64)
    neg_t = np.asarray(neg_t).astype(np.int64)
    ent = np.ascontiguousarray(np.asarray(entity_embed, dtype=np.float32))
    re_ = np.ascontiguousarray(np.asarray(relation_embed, dtype=np.float32))
    rw = np.ascontiguousarray(np.asarray(relation_weight, dtype=np.float32))

    nb, maps = _plan(h, r, pos_t, neg_t, rw, re_)
    if nb not in _cache:
        _cache[nb] = _build(nb)
    nc = _cache[nb]

    in_maps = []
    for c in range(N_CORES):
        m = maps[c]
        in_maps.append(
            {
                "ent": ent,
                "idx3": m["idx3"],
                "mbias": m["mbias"],
                "wsl": _to_bf16(m["wsl"]),
                "selr": _to_bf16(m["selr"]),
                "idn": _to_bf16(m["idn"]),
            }
        )

    # host-side regularizer (1e-5-scaled, ~1e-4 of the loss; fp64 exact)
    he = ent[h].astype(np.float64)
    pe = ent[pos_t].astype(np.float64)
    ne = ent[neg_t].astype(np.float64)
    rr = re_[r].astype(np.float64)
    reg = (
        np.mean(np.sum(he * he, axis=1)) / 2.0
        + np.mean(np.sum(rr * rr, axis=1)) / 2.0
        + np.mean(np.sum(pe * pe, axis=1)) / 2.0
        + np.mean(np.sum(ne * ne, axis=1)) / 2.0
    )

    if os.environ.get("KGE_SIM"):
        from concourse.bass_interp import CoreSim

        total = 0.0
        ncores = int(os.environ.get("KGE_SIM_CORES", N_CORES))
        for c in range(ncores):
            sim = CoreSim(nc, trace=False)
            for name, arr in in_maps[c].items():
                sim.tensor(name)[:] = arr
            sim.simulate()
            total += float(sim.tensor("out")[0, 0])
        if ncores < N_CORES:
            return np.float32(total)  # partial, dev only
        return np.float32(total / M + LAM * reg)

    from concourse.bass_utils import run_bass_kernel_spmd

    res = run_bass_kernel_spmd(nc, in_maps, core_ids=list(range(N_CORES)))
    total = sum(float(res.results[c]["out"][0, 0]) for c in range(N_CORES))
    return np.float32(total / M + LAM * reg)


# revision 8
# speedup vs baseline: 1.6655x; 1.0029x over previous
"""KGE (TransR-style) loss kernel for Trainium2, 8 NeuronCores.

Strategy (v2):
  - Host: sort the M=8192 triples by relation id, pad each relation's
    segment to 64-row runs -> ~80 blocks of 128 rows where every block is
    exactly two single-relation 64-row runs.  10 blocks/core, one SPMD
    program for all 8 cores.  W tables are packed one slot per run
    (duplicated as needed) so all device-side APs are static.
  - Device (per core):
      * ONE indirect DMA per half gathers all h/pos/neg entity rows into
        X = [H|P|N] per block  (2 SWDGE instructions instead of 36: the
        994ns/instr descriptor-generation overhead dominated the old
        kernel)
      * D_pos = H - P, D_neg = H - N on DVE as two wide strided
        instructions, f32 in -> bf16 out
      * PE transposes D -> D^T in bf16 (1 cyc/row), 8 tiles per PSUM
        bank, one wide ACT copy per bank to SBUF
      * per block: V = [D_pos@W | D_neg@W] + r in one PSUM bank via
        5 bf16 matmuls (one K=16 one-hot matmul adds r to both halves,
        then 4 64-row run matmuls with the run's W slot)
      * ACT Square V -> bf16 squares in SBUF (one instr per block)
      * ONE segmented DVE reduce (axis=X over [128, 2NB, 128]) gives all
        scores; softplus tail batched over [128, NB]
  - Final: free-dim reduce + ones-matmul partition reduce -> one f32 per
    core; host sums the 8 partials, divides by M and adds the (tiny,
    1e-5-scaled) embedding-norm regularizer computed on host in fp64.
"""

import os
from contextlib import ExitStack

import numpy as np

import concourse.bass as bass
import concourse.tile as tile
from concourse import bacc, mybir

M = 8192
E = 128
N_ENT = 500000
N_REL = 64
LAM = 1e-5
P = 128
RUN = 64
KSEL = 16
N_CORES = 8
PAD_BIAS = -30000.0

f32 = mybir.dt.float32
bf16 = mybir.dt.bfloat16
i32 = mybir.dt.int32

_cache = {}


def _build(NB: int):
    """Build + compile the single-core SPMD program for NB blocks/core."""
    nc = bacc.Bacc(
        "TRN2",
        target_bir_lowering=False,
        debug=False,
        num_devices=N_CORES,
        # default 16KiB ring = 1024 SWDGE descriptors; the batched gathers
        # need ~4k descriptors in flight or they serialize on ring drain
        dynamic_dma_scratch_size=65536,
    )

    ent = nc.dram_tensor("ent", (N_ENT, E), f32, kind="ExternalInput").ap()
    idx3 = nc.dram_tensor("idx3", (P, NB * 3), i32, kind="ExternalInput").ap()
    # mbias columns + ones column for the final partition reduce
    mbias = nc.dram_tensor("mbias", (P, NB + 1), f32, kind="ExternalInput").ap()
    wsl = nc.dram_tensor("wsl", (P, 2 * NB * P), bf16, kind="ExternalInput").ap()
    # sel one-hot [KSEL, NB*P] followed by r2 = [r|r] [KSEL, 2P]
    selr = nc.dram_tensor("selr", (KSEL, NB * P + 2 * P), bf16, kind="ExternalInput").ap()
    idn = nc.dram_tensor("idn", (P, P), bf16, kind="ExternalInput").ap()
    out = nc.dram_tensor("out", (1, 1), f32, kind="ExternalOutput").ap()

    # split the gather into chunks of blocks for DMA/compute pipelining
    NG = 3 if NB >= 3 else NB
    base, rem = divmod(NB, NG)
    chunks = []
    b0 = 0
    for g in range(NG):
        cg = base + (1 if g < rem else 0)
        chunks.append((b0, cg))
        b0 += cg

    with tile.TileContext(nc) as tc, ExitStack() as ctx:
        const = ctx.enter_context(tc.tile_pool(name="const", bufs=1))
        dtp = ctx.enter_context(tc.tile_pool(name="dtp", bufs=2, space="PSUM"))
        vp = ctx.enter_context(tc.tile_pool(name="vp", bufs=4, space="PSUM"))
        fp = ctx.enter_context(tc.tile_pool(name="fp", bufs=1, space="PSUM"))

        # ---- input loads ----
        idx_sb = const.tile([P, 3 * NB], i32)
        nc.sync.dma_start(out=idx_sb[:], in_=idx3[:])
        w_sb = const.tile([P, 2 * NB * P], bf16)
        nc.sync.dma_start(out=w_sb[:], in_=wsl[:])
        selr_sb = const.tile([KSEL, NB * P + 2 * P], bf16)
        nc.sync.dma_start(out=selr_sb[:], in_=selr[:])
        mb_sb = const.tile([P, NB + 1], f32)
        nc.sync.dma_start(out=mb_sb[:], in_=mbias[:])
        idn_sb = const.tile([P, P], bf16)
        nc.sync.dma_start(out=idn_sb[:], in_=idn[:])

        # dummy activations so the act-table pass picks ONE table serving
        # {Ln, Exp, Abs, Relu, Square, Copy} (natural_log_exp_and_others)
        # instead of loading a second table mid-kernel (1283ns each)
        dum = const.tile([1, 1], f32)
        nc.gpsimd.memset(dum[:], 1.0)
        dum2 = const.tile([1, 1], f32)
        nc.scalar.activation(
            out=dum2[:], in_=dum[:], func=mybir.ActivationFunctionType.Ln
        )
        nc.scalar.activation(
            out=dum2[:], in_=dum[:], func=mybir.ActivationFunctionType.Exp
        )

        xs = []
        for g, (b0, cg) in enumerate(chunks):
            x = const.tile([P, cg * 3 * E], f32)
            nc.gpsimd.indirect_dma_start(
                out=x[:],
                out_offset=None,
                in_=ent[:],
                in_offset=bass.IndirectOffsetOnAxis(
                    ap=idx_sb[:, 3 * b0 : 3 * (b0 + cg)], axis=0
                ),
            )
            xs.append(x)

        # D_pos/D_neg per chunk: wide strided subtract, f32 -> bf16
        dps, dns = [], []
        for g, (b0, cg) in enumerate(chunks):
            x3 = xs[g][:].rearrange("p (b x) -> p b x", x=3 * E)
            dp = const.tile([P, cg * E], bf16)
            nc.vector.tensor_tensor(
                out=dp[:].rearrange("p (b x) -> p b x", x=E),
                in0=x3[:, :, 0:E],
                in1=x3[:, :, E : 2 * E],
                op=mybir.AluOpType.subtract,
            )
            dn = const.tile([P, cg * E], bf16)
            nc.vector.tensor_tensor(
                out=dn[:].rearrange("p (b x) -> p b x", x=E),
                in0=x3[:, :, 0:E],
                in1=x3[:, :, 2 * E : 3 * E],
                op=mybir.AluOpType.subtract,
            )
            dps.append(dp)
            dns.append(dn)

        # transposes: per chunk, groups of up to 4 blocks -> one PSUM bank
        # (8 bf16 [128,128] tiles), one wide ACT copy to SBUF per bank
        dT = const.tile([P, 2 * NB * E], bf16)  # block b: pos at 2b, neg at 2b+1
        sq_all = const.tile([P, 2 * NB * E], bf16)
        scores = const.tile([P, 2 * NB], f32)

        r2 = selr_sb[:, NB * P : NB * P + 2 * P]

        for g, (b0, cg) in enumerate(chunks):
            for g4 in range(0, cg, 4):
                n4 = min(4, cg - g4)
                dt_ps = dtp.tile([P, 8 * E], bf16, tag="dt")
                for i in range(n4):
                    bl = g4 + i  # block local to chunk
                    nc.tensor.transpose(
                        out=dt_ps[:, (2 * i) * E : (2 * i + 1) * E],
                        in_=dps[g][:, bl * E : (bl + 1) * E],
                        identity=idn_sb[:],
                    )
                    nc.tensor.transpose(
                        out=dt_ps[:, (2 * i + 1) * E : (2 * i + 2) * E],
                        in_=dns[g][:, bl * E : (bl + 1) * E],
                        identity=idn_sb[:],
                    )
                dst0 = 2 * (b0 + g4) * E
                nc.scalar.copy(
                    dT[:, dst0 : dst0 + 2 * n4 * E], dt_ps[:, 0 : 2 * n4 * E]
                )

            # matmuls + squares for this chunk's blocks
            for bl in range(cg):
                b = b0 + bl
                v_ps = vp.tile([P, 512], f32, tag="v")
                # r add: one K=16 one-hot matmul covers pos and neg halves
                nc.tensor.matmul(
                    out=v_ps[:, 0:256],
                    lhsT=selr_sb[:, b * P : (b + 1) * P],
                    rhs=r2,
                    start=True,
                    stop=False,
                )
                dposT = dT[:, (2 * b) * E : (2 * b + 1) * E]
                dnegT = dT[:, (2 * b + 1) * E : (2 * b + 2) * E]
                for u in range(2):
                    wslot = w_sb[:, (2 * b + u) * P : (2 * b + u + 1) * P]
                    rows = slice(u * RUN, (u + 1) * RUN)
                    nc.tensor.matmul(
                        out=v_ps[rows, 0:128],
                        lhsT=dposT[:, rows],
                        rhs=wslot,
                        start=False,
                        stop=False,
                    )
                    nc.tensor.matmul(
                        out=v_ps[rows, 128:256],
                        lhsT=dnegT[:, rows],
                        rhs=wslot,
                        start=False,
                        stop=True,
                    )
                nc.scalar.activation(
                    out=sq_all[:, b * 256 : (b + 1) * 256],
                    in_=v_ps[:, 0:256],
                    func=mybir.ActivationFunctionType.Square,
                )

            # per-chunk segmented reduce so it overlaps later chunks
            nc.vector.reduce_sum(
                out=scores[:, 2 * b0 : 2 * (b0 + cg)],
                in_=sq_all[:, b0 * 256 : (b0 + cg) * 256].rearrange(
                    "p (s x) -> p s x", x=E
                ),
                axis=mybir.AxisListType.X,
            )

        # ---- batched tail ----
        s2 = scores[:].rearrange("p (b two) -> p b two", two=2)
        dcol = const.tile([P, NB], f32)
        nc.vector.tensor_tensor(
            out=dcol[:].unsqueeze(2),
            in0=s2[:, :, 1:2],
            in1=s2[:, :, 0:1],
            op=mybir.AluOpType.subtract,
        )
        y = const.tile([P, NB], f32)
        nc.vector.tensor_scalar_mul(out=y[:], in0=dcol[:], scalar1=0.5)
        nc.vector.tensor_tensor(
            out=y[:], in0=y[:], in1=mb_sb[:, 0:NB], op=mybir.AluOpType.add
        )
        # softplus(y) = relu(y) + ln(1 + exp(-|y|))
        t_abs = const.tile([P, NB], f32)
        nc.scalar.activation(
            out=t_abs[:], in_=y[:], func=mybir.ActivationFunctionType.Abs
        )
        t_exp = const.tile([P, NB], f32)
        nc.scalar.activation(
            out=t_exp[:], in_=t_abs[:], func=mybir.ActivationFunctionType.Exp,
            scale=-1.0,
        )
        t_ln = const.tile([P, NB], f32)
        nc.scalar.activation(
            out=t_ln[:], in_=t_exp[:], func=mybir.ActivationFunctionType.Ln,
            bias=1.0,
        )
        t_relu = const.tile([P, NB], f32)
        nc.scalar.activation(
            out=t_relu[:], in_=y[:], func=mybir.ActivationFunctionType.Relu
        )
        acc = const.tile([P, NB], f32)
        nc.vector.tensor_tensor(
            out=acc[:], in0=t_ln[:], in1=t_relu[:], op=mybir.AluOpType.add
        )
        tcol = const.tile([P, 1], f32)
        nc.vector.reduce_sum(out=tcol[:], in_=acc[:], axis=mybir.AxisListType.X)
        fin_ps = fp.tile([1, 1], f32, tag="fin")
        nc.tensor.matmul(
            out=fin_ps[:], lhsT=tcol[:], rhs=mb_sb[:, NB : NB + 1],
            start=True, stop=True,
        )
        fin_sb = const.tile([1, 1], f32)
        nc.scalar.copy(fin_sb[:], fin_ps[:])
        nc.sync.dma_start(out=out[:], in_=fin_sb[:])

    nc.compile()
    return nc


def _plan(h, r, pos_t, neg_t, relation_weight, relation_embed):
    """Sort by relation, pad to 64-row single-relation runs, split 8 ways."""
    order = np.argsort(r, kind="stable")
    counts = np.bincount(r, minlength=N_REL)
    rows_parts, rel_parts = [], []
    pos = 0
    for k in range(N_REL):
        c = int(counts[k])
        ids = order[pos : pos + c]
        pos += c
        if c == 0:
            continue
        n_slots = -(-c // RUN) * RUN
        arr = np.full(n_slots, -1, np.int64)
        arr[:c] = ids
        rows_parts.append(arr)
        rel_parts.append(np.full(n_slots, k, np.int64))
    rows = np.concatenate(rows_parts)
    rels = np.concatenate(rel_parts)
    t0 = len(rows)
    nb = max(1, -(-t0 // (P * N_CORES)))
    t = nb * P * N_CORES
    if t > t0:
        rows = np.concatenate([rows, np.full(t - t0, -1, np.int64)])
        rels = np.concatenate([rels, np.zeros(t - t0, np.int64)])

    iden = np.eye(P, dtype=np.float32)

    maps = []
    for c in range(N_CORES):
        sl_rows = rows[c * nb * P : (c + 1) * nb * P]
        sl_rels = rels[c * nb * P : (c + 1) * nb * P]
        core_rels = []
        for k in sl_rels[::RUN]:
            if k not in core_rels:
                core_rels.append(int(k))
        assert len(core_rels) <= KSEL, f"core {c} has {len(core_rels)} relations"
        rel_slot = {k: i for i, k in enumerate(core_rels)}

        idx3 = np.zeros((P, nb, 3), np.int32)
        mb = np.full((P, nb + 1), PAD_BIAS, np.float32)
        mb[:, nb] = 1.0  # ones column for the final reduce
        wv = np.zeros((P, 2 * nb, P), np.float32)
        sel = np.zeros((KSEL, nb * P + 2 * P), np.float32)
        for b in range(nb):
            for u in range(2):
                k = int(sl_rels[(2 * b + u) * RUN])
                wv[:, 2 * b + u, :] = relation_weight[k]
            for p in range(P):
                s = b * P + p
                rid = sl_rows[s]
                if rid >= 0:
                    idx3[p, b, 0] = h[rid]
                    idx3[p, b, 1] = pos_t[rid]
                    idx3[p, b, 2] = neg_t[rid]
                    mb[p, b] = 0.0
                    sel[rel_slot[int(sl_rels[s])], s] = 1.0
        for i, k in enumerate(core_rels):
            sel[i, nb * P : nb * P + P] = relation_embed[k]
            sel[i, nb * P + P : nb * P + 2 * P] = relation_embed[k]
        maps.append(
            {
                "idx3": idx3.reshape(P, nb * 3),
                "mbias": mb,
                "wsl": wv.reshape(P, 2 * nb * P),
                "selr": sel,
                "idn": iden,
            }
        )
    return nb, maps


def _to_bf16(a):
    import ml_dtypes

    return np.asarray(a, dtype=np.float32).astype(ml_dtypes.bfloat16)


def kernel(h, r, pos_t, neg_t, entity_embed, relation_embed, relation_weight):
    h = np.asarray(h).astype(np.int64)
    r = np.asarray(r).astype(np.int64)
    pos_t = np.asarray(pos_t).astype(np.int64)
    neg_t = np.asarray(neg_t).astype(np.int64)
    ent = np.ascontiguousarray(np.asarray(entity_embed, dtype=np.float32))
    re_ = np.ascontiguousarray(np.asarray(relation_embed, dtype=np.float32))
    rw = np.ascontiguousarray(np.asarray(relation_weight, dtype=np.float32))

    nb, maps = _plan(h, r, pos_t, neg_t, rw, re_)
    if nb not in _cache:
        _cache[nb] = _build(nb)
    nc = _cache[nb]

    in_maps = []
    for c in range(N_CORES):
        m = maps[c]
        in_maps.append(
            {
                "ent": ent,
                "idx3": m["idx3"],
                "mbias": m["mbias"],
                "wsl": _to_bf16(m["wsl"]),
                "selr": _to_bf16(m["selr"]),
                "idn": _to_bf16(m["idn"]),
            }
        )

    # host-side regularizer (1e-5-scaled, ~1e-4 of the loss; fp64 exact)
    he = ent[h].astype(np.float64)
    pe = ent[pos_t].astype(np.float64)
    ne = ent[neg_t].astype(np.float64)
    rr = re_[r].astype(np.float64)
    reg = (
        np.mean(np.sum(he * he, axis=1)) / 2.0
        + np.mean(np.sum(rr * rr, axis=1)) / 2.0
        + np.mean(np.sum(pe * pe, axis=1)) / 2.0
        + np.mean(np.sum(ne * ne, axis=1)) / 2.0
    )

    if os.environ.get("KGE_SIM"):
        from concourse.bass_interp import CoreSim

        total = 0.0
        ncores = int(os.environ.get("KGE_SIM_CORES", N_CORES))
        for c in range(ncores):
            sim = CoreSim(nc, trace=False)
            for name, arr in in_maps[c].items():
                sim.tensor(name)[:] = arr
            sim.simulate()
            total += float(sim.tensor("out")[0, 0])
        if ncores < N_CORES:
            return np.float32(total)  # partial, dev only
        return np.float32(total / M + LAM * reg)

    from concourse.bass_utils import run_bass_kernel_spmd

    res = run_bass_kernel_spmd(nc, in_maps, core_ids=list(range(N_CORES)))
    total = sum(float(res.results[c]["out"][0, 0]) for c in range(N_CORES))
    return np.float32(total / M + LAM * reg)


# revision 31
# speedup vs baseline: 2.2380x; 1.3437x over previous
"""KGE (TransR-style) loss kernel for Trainium2, 8 NeuronCores.

Strategy (v2):
  - Host: sort the M=8192 triples by relation id, pad each relation's
    segment to 64-row runs -> ~80 blocks of 128 rows where every block is
    exactly two single-relation 64-row runs.  10 blocks/core, one SPMD
    program for all 8 cores.  W tables are packed one slot per run
    (duplicated as needed) so all device-side APs are static.
  - Device (per core):
      * ONE indirect DMA per half gathers all h/pos/neg entity rows into
        X = [H|P|N] per block  (2 SWDGE instructions instead of 36: the
        994ns/instr descriptor-generation overhead dominated the old
        kernel)
      * D_pos = H - P, D_neg = H - N on DVE as two wide strided
        instructions, f32 in -> bf16 out
      * PE transposes D -> D^T in bf16 (1 cyc/row), 8 tiles per PSUM
        bank, one wide ACT copy per bank to SBUF
      * per block: V = [D_pos@W | D_neg@W] + r in one PSUM bank via
        5 bf16 matmuls (one K=16 one-hot matmul adds r to both halves,
        then 4 64-row run matmuls with the run's W slot)
      * ACT Square V -> bf16 squares in SBUF (one instr per block)
      * ONE segmented DVE reduce (axis=X over [128, 2NB, 128]) gives all
        scores; softplus tail batched over [128, NB]
  - Final: free-dim reduce + ones-matmul partition reduce -> one f32 per
    core; host sums the 8 partials, divides by M and adds the (tiny,
    1e-5-scaled) embedding-norm regularizer computed on host in fp64.
"""

import os
from contextlib import ExitStack

import numpy as np

import concourse.bass as bass
import concourse.tile as tile
from concourse import bacc, mybir

M = 8192
E = 128
N_ENT = 500000
N_REL = 64
LAM = 1e-5
P = 128
RUN = 64
KSEL = 16
N_CORES = 8
PAD_BIAS = -30000.0

f32 = mybir.dt.float32
bf16 = mybir.dt.bfloat16
i32 = mybir.dt.int32

# degree-4 near-minimax fit of ln(1+t) on [0,1], max err 1.4e-4
LN1P_C = (
    0.0001408330802882829, 0.995434095113967, -0.46409135123177386,
    0.21642895969231737, -0.05486825942200259,
)


def _softplus0_f32():
    """softplus(0) exactly as the device computes it (f32 ops, t=1)."""
    f = np.float32
    t = f(1.0)
    t2 = f(t * t)
    u1 = f(f(t * f(LN1P_C[1])) + f(LN1P_C[0]))
    u2 = f(f(t * f(LN1P_C[3])) + f(LN1P_C[2]))
    w4 = f(t2 * f(LN1P_C[4]))
    u2 = f(u2 + w4)
    u2 = f(u2 * t2)
    return float(f(u1 + u2))


_cache = {}


def _build(NB: int):
    """Build + compile the single-core SPMD program for NB blocks/core."""
    nc = bacc.Bacc(
        "TRN2",
        target_bir_lowering=False,
        debug=False,
        num_devices=N_CORES,
        # default 16KiB ring = 1024 SWDGE descriptors; the batched gathers
        # need ~4k descriptors in flight or they serialize on ring drain
        dynamic_dma_scratch_size=65536,
    )

    assert NB % 2 == 0, "block pairing assumes even NB"
    NQ = NB // 2  # block pairs; each pair shares one PSUM bank

    ent = nc.dram_tensor("ent", (N_ENT, E), f32, kind="ExternalInput").ap()
    idx3 = nc.dram_tensor("idx3", (P, NB * 3), i32, kind="ExternalInput").ap()
    wsl = nc.dram_tensor("wsl", (P, 2 * NB * P), bf16, kind="ExternalInput").ap()
    # paired sel one-hot [2*KSEL, NQ*P] followed by r4 [2*KSEL, 4P]
    selr = nc.dram_tensor(
        "selr", (2 * KSEL, NQ * P + 4 * P), bf16, kind="ExternalInput"
    ).ap()
    idn = nc.dram_tensor("idn", (P, P), bf16, kind="ExternalInput").ap()
    # per-(row,block) softplus values; host does the final sum
    out = nc.dram_tensor("out", (P, NB), f32, kind="ExternalOutput").ap()

    # split the gather into chunks of block pairs for DMA/compute
    # pipelining; last chunk smallest so the tail chase is short
    if NQ >= 3:
        sizes_q = [NQ - NQ // 2 - 1, NQ // 2, 1]
    else:
        sizes_q = [NQ]
    chunks = []
    q0 = 0
    for cq in sizes_q:
        chunks.append((2 * q0, 2 * cq))
        q0 += cq

    with tile.TileContext(nc) as tc, ExitStack() as ctx:
        const = ctx.enter_context(tc.tile_pool(name="const", bufs=1))
        dtp = ctx.enter_context(tc.tile_pool(name="dtp", bufs=2, space="PSUM"))
        vp = ctx.enter_context(tc.tile_pool(name="vp", bufs=4, space="PSUM"))

        # ---- input loads ----
        w_sb = const.tile([P, 2 * NB * P], bf16)
        nc.sync.dma_start(out=w_sb[:], in_=wsl[:])
        selr_sb = const.tile([2 * KSEL, NQ * P + 4 * P], bf16)
        nc.sync.dma_start(out=selr_sb[:], in_=selr[:])
        idn_sb = const.tile([P, P], bf16)
        nc.sync.dma_start(out=idn_sb[:], in_=idn[:])

        # gather offsets read straight from DRAM: skips the ~2.9us
        # idx-load latency before SWDGE descriptor generation can start
        xs = []
        for g, (b0, cg) in enumerate(chunks):
            x = const.tile([P, cg * 3 * E], f32, tag=f"x{g}")
            nc.gpsimd.indirect_dma_start(
                out=x[:],
                out_offset=None,
                in_=ent[:],
                in_offset=bass.IndirectOffsetOnAxis(
                    ap=idx3[:, 3 * b0 : 3 * (b0 + cg)], axis=0
                ),
            )
            xs.append(x)

        # D_pos/D_neg per chunk: wide strided subtract, f32 -> bf16
        dps, dns = [], []
        for g, (b0, cg) in enumerate(chunks):
            x3 = xs[g][:].rearrange("p (b x) -> p b x", x=3 * E)
            dp = const.tile([P, cg * E], bf16, tag=f"dp{g}")
            nc.vector.tensor_tensor(
                out=dp[:].rearrange("p (b x) -> p b x", x=E),
                in0=x3[:, :, 0:E],
                in1=x3[:, :, E : 2 * E],
                op=mybir.AluOpType.subtract,
            )
            dn = const.tile([P, cg * E], bf16, tag=f"dn{g}")
            nc.vector.tensor_tensor(
                out=dn[:].rearrange("p (b x) -> p b x", x=E),
                in0=x3[:, :, 0:E],
                in1=x3[:, :, 2 * E : 3 * E],
                op=mybir.AluOpType.subtract,
            )
            dps.append(dp)
            dns.append(dn)

        # transposes: per chunk -> one PSUM bank per 2 pairs, one ACT copy
        # per pair (4 bf16 [128,128] D^T tiles) to SBUF
        dT = const.tile([P, 2 * NB * E], bf16)  # block b: pos at 2b, neg at 2b+1
        sq_all = const.tile([P, 2 * NB * E], bf16)
        scores = const.tile([P, 2 * NB], f32)

        r4 = selr_sb[:, NQ * P : NQ * P + 4 * P]

        for g, (b0, cg) in enumerate(chunks):
            for g4 in range(0, cg, 4):
                n4 = min(4, cg - g4)
                dt_ps = dtp.tile([P, 8 * E], bf16, tag="dt")
                for i in range(n4):
                    bl = g4 + i  # block local to chunk
                    nc.tensor.transpose(
                        out=dt_ps[:, (2 * i) * E : (2 * i + 1) * E],
                        in_=dps[g][:, bl * E : (bl + 1) * E],
                        identity=idn_sb[:],
                    )
                    nc.tensor.transpose(
                        out=dt_ps[:, (2 * i + 1) * E : (2 * i + 2) * E],
                        in_=dns[g][:, bl * E : (bl + 1) * E],
                        identity=idn_sb[:],
                    )
                # one copy per pair (half the bank)
                for i in range(0, n4, 2):
                    dst0 = 2 * (b0 + g4 + i) * E
                    w_ = 2 * min(2, n4 - i) * E
                    nc.scalar.copy(
                        dT[:, dst0 : dst0 + w_],
                        dt_ps[:, 2 * i * E : 2 * i * E + w_],
                    )

            # matmuls + squares + reduce per block pair
            for ql in range(cg // 2):
                q = b0 // 2 + ql
                bA, bB = 2 * q, 2 * q + 1
                v_ps = vp.tile([P, 512], f32, tag="v")
                # r add for both blocks of the pair: K=32 one-hot matmul.
                # start+stop here; the partition-sliced run matmuls below
                # accumulate with skip_group_check (the sim's zero-region
                # bookkeeping mishandles partition-offset outputs; the
                # accumulation itself is exact)
                nc.tensor.matmul(
                    out=v_ps[:],
                    lhsT=selr_sb[:, q * P : (q + 1) * P],
                    rhs=r4,
                    start=True,
                    stop=True,
                )
                for j, b in enumerate((bA, bB)):
                    dposT = dT[:, (2 * b) * E : (2 * b + 1) * E]
                    dnegT = dT[:, (2 * b + 1) * E : (2 * b + 2) * E]
                    c4 = 256 * j
                    for u in range(2):
                        wslot = w_sb[:, (2 * b + u) * P : (2 * b + u + 1) * P]
                        rows = slice(u * RUN, (u + 1) * RUN)
                        nc.tensor.matmul(
                            out=v_ps[rows, c4 : c4 + 128],
                            lhsT=dposT[:, rows],
                            rhs=wslot,
                            start=False,
                            stop=False,
                            skip_group_check=True,
                        )
                        nc.tensor.matmul(
                            out=v_ps[rows, c4 + 128 : c4 + 256],
                            lhsT=dnegT[:, rows],
                            rhs=wslot,
                            start=False,
                            stop=False,
                            skip_group_check=True,
                        )
                nc.scalar.activation(
                    out=sq_all[:, q * 512 : (q + 1) * 512],
                    in_=v_ps[:],
                    func=mybir.ActivationFunctionType.Square,
                )
                # per-pair segmented reduce overlaps later pairs
                nc.vector.reduce_sum(
                    out=scores[:, 4 * q : 4 * q + 4],
                    in_=sq_all[:, q * 512 : (q + 1) * 512].rearrange(
                        "p (s x) -> p s x", x=E
                    ),
                    axis=mybir.AxisListType.X,
                )

        # ---- batched tail ----
        # W and r were pre-scaled by sqrt(0.5) on host, so scores are
        # already halved: y = (neg - pos) directly.  Pad slots have D=0 and
        # sel=0 so y=0 exactly; the host subtracts n_pad*softplus(0).
        s2 = scores[:].rearrange("p (b two) -> p b two", two=2)
        y = const.tile([P, NB], f32)
        nc.vector.tensor_tensor(
            out=y[:].unsqueeze(2),
            in0=s2[:, :, 1:2],
            in1=s2[:, :, 0:1],
            op=mybir.AluOpType.subtract,
        )
        # softplus(y) = relu(y) + ln(1 + t), t = exp(-|y|).  ln(1+t) is a
        # degree-4 polynomial on DVE (max err 1.4e-4 on [0,1]) so the ACT
        # engine never needs the Ln table (saves a 1283ns table reload)
        t_abs = const.tile([P, NB], f32)
        nc.scalar.activation(
            out=t_abs[:], in_=y[:], func=mybir.ActivationFunctionType.Abs
        )
        t = const.tile([P, NB], f32)
        nc.scalar.activation(
            out=t[:], in_=t_abs[:], func=mybir.ActivationFunctionType.Exp,
            scale=-1.0,
        )
        t2 = const.tile([P, NB], f32)
        nc.vector.tensor_tensor(
            out=t2[:], in0=t[:], in1=t[:], op=mybir.AluOpType.mult
        )
        u1 = const.tile([P, NB], f32)
        nc.vector.tensor_scalar(
            out=u1[:], in0=t[:], scalar1=LN1P_C[1], scalar2=LN1P_C[0],
            op0=mybir.AluOpType.mult, op1=mybir.AluOpType.add,
        )
        u2 = const.tile([P, NB], f32)
        nc.vector.tensor_scalar(
            out=u2[:], in0=t[:], scalar1=LN1P_C[3], scalar2=LN1P_C[2],
            op0=mybir.AluOpType.mult, op1=mybir.AluOpType.add,
        )
        w4 = const.tile([P, NB], f32)
        nc.vector.tensor_scalar_mul(out=w4[:], in0=t2[:], scalar1=LN1P_C[4])
        nc.vector.tensor_tensor(
            out=u2[:], in0=u2[:], in1=w4[:], op=mybir.AluOpType.add
        )
        nc.vector.tensor_tensor(
            out=u2[:], in0=u2[:], in1=t2[:], op=mybir.AluOpType.mult
        )
        t_relu = const.tile([P, NB], f32)
        nc.scalar.activation(
            out=t_relu[:], in_=y[:], func=mybir.ActivationFunctionType.Relu
        )
        acc = const.tile([P, NB], f32)
        nc.vector.tensor_tensor(
            out=acc[:], in0=u1[:], in1=u2[:], op=mybir.AluOpType.add
        )
        nc.vector.tensor_tensor(
            out=acc[:], in0=acc[:], in1=t_relu[:], op=mybir.AluOpType.add
        )
        nc.sync.dma_start(out=out[:], in_=acc[:])

    nc.compile()
    return nc


def _plan(h, r, pos_t, neg_t, relation_weight, relation_embed):
    """Sort by relation, pad to 64-row single-relation runs, split 8 ways."""
    order = np.argsort(r, kind="stable")
    counts = np.bincount(r, minlength=N_REL)
    rows_parts, rel_parts = [], []
    pos = 0
    for k in range(N_REL):
        c = int(counts[k])
        ids = order[pos : pos + c]
        pos += c
        if c == 0:
            continue
        n_slots = -(-c // RUN) * RUN
        arr = np.full(n_slots, -1, np.int64)
        arr[:c] = ids
        rows_parts.append(arr)
        rel_parts.append(np.full(n_slots, k, np.int64))
    rows = np.concatenate(rows_parts)
    rels = np.concatenate(rel_parts)
    t0 = len(rows)
    nb = max(1, -(-t0 // (P * N_CORES)))
    t = nb * P * N_CORES
    if t > t0:
        rows = np.concatenate([rows, np.full(t - t0, -1, np.int64)])
        rels = np.concatenate([rels, np.zeros(t - t0, np.int64)])

    iden = np.eye(P, dtype=np.float32)

    maps = []
    for c in range(N_CORES):
        sl_rows = rows[c * nb * P : (c + 1) * nb * P]
        sl_rels = rels[c * nb * P : (c + 1) * nb * P]
        core_rels = []
        for k in sl_rels[::RUN]:
            if k not in core_rels:
                core_rels.append(int(k))
        assert len(core_rels) <= KSEL, f"core {c} has {len(core_rels)} relations"
        rel_slot = {k: i for i, k in enumerate(core_rels)}

        nq = nb // 2
        idx3 = np.zeros((P, nb, 3), np.int32)
        wv = np.zeros((P, 2 * nb, P), np.float32)
        sel = np.zeros((2 * KSEL, nq * P + 4 * P), np.float32)
        s5 = np.float32(np.sqrt(0.5))
        for b in range(nb):
            for u in range(2):
                k = int(sl_rels[(2 * b + u) * RUN])
                wv[:, 2 * b + u, :] = relation_weight[k] * s5
            q, j = divmod(b, 2)
            for p in range(P):
                s = b * P + p
                rid = sl_rows[s]
                if rid >= 0:
                    idx3[p, b, 0] = h[rid]
                    idx3[p, b, 1] = pos_t[rid]
                    idx3[p, b, 2] = neg_t[rid]
                    sel[j * KSEL + rel_slot[int(sl_rels[s])], q * P + p] = 1.0
        # r4 = [[r|r], 0; 0, [r|r]] so one K=32 matmul adds r to both
        # blocks of a pair (pos and neg halves)
        for i, k in enumerate(core_rels):
            rrow = relation_embed[k] * s5
            for rep in range(2):
                sel[i, nq * P + rep * P : nq * P + (rep + 1) * P] = rrow
                sel[KSEL + i, nq * P + (2 + rep) * P : nq * P + (3 + rep) * P] = rrow
        maps.append(
            {
                "idx3": idx3.reshape(P, nb * 3),
                "wsl": wv.reshape(P, 2 * nb * P),
                "selr": sel,
                "idn": iden,
            }
        )
    n_pad = len(rows) - M
    return nb, n_pad, maps


def _to_bf16(a):
    import ml_dtypes

    return np.asarray(a, dtype=np.float32).astype(ml_dtypes.bfloat16)


def kernel(h, r, pos_t, neg_t, entity_embed, relation_embed, relation_weight):
    h = np.asarray(h).astype(np.int64)
    r = np.asarray(r).astype(np.int64)
    pos_t = np.asarray(pos_t).astype(np.int64)
    neg_t = np.asarray(neg_t).astype(np.int64)
    ent = np.ascontiguousarray(np.asarray(entity_embed, dtype=np.float32))
    re_ = np.ascontiguousarray(np.asarray(relation_embed, dtype=np.float32))
    rw = np.ascontiguousarray(np.asarray(relation_weight, dtype=np.float32))

    nb, n_pad, maps = _plan(h, r, pos_t, neg_t, rw, re_)
    if nb not in _cache:
        _cache[nb] = _build(nb)
    nc = _cache[nb]

    in_maps = []
    for c in range(N_CORES):
        m = maps[c]
        in_maps.append(
            {
                "ent": ent,
                "idx3": m["idx3"],
                "wsl": _to_bf16(m["wsl"]),
                "selr": _to_bf16(m["selr"]),
                "idn": _to_bf16(m["idn"]),
            }
        )

    # host-side regularizer (1e-5-scaled, ~1e-4 of the loss; fp64 exact)
    he = ent[h].astype(np.float64)
    pe = ent[pos_t].astype(np.float64)
    ne = ent[neg_t].astype(np.float64)
    rr = re_[r].astype(np.float64)
    reg = (
        np.mean(np.sum(he * he, axis=1)) / 2.0
        + np.mean(np.sum(rr * rr, axis=1)) / 2.0
        + np.mean(np.sum(pe * pe, axis=1)) / 2.0
        + np.mean(np.sum(ne * ne, axis=1)) / 2.0
    )

    pad_corr = n_pad * _softplus0_f32()

    if os.environ.get("KGE_SIM"):
        from concourse.bass_interp import CoreSim

        total = 0.0
        ncores = int(os.environ.get("KGE_SIM_CORES", N_CORES))
        for c in range(ncores):
            sim = CoreSim(nc, trace=False)
            for name, arr in in_maps[c].items():
                sim.tensor(name)[:] = arr
            sim.simulate()
            total += float(np.sum(np.asarray(sim.tensor("out"), np.float64)))
        if ncores < N_CORES:
            return np.float32(total)  # partial, dev only
        return np.float32((total - pad_corr) / M + LAM * reg)

    from concourse.bass_utils import run_bass_kernel_spmd

    res = run_bass_kernel_spmd(nc, in_maps, core_ids=list(range(N_CORES)))
    total = sum(
        float(np.sum(np.asarray(res.results[c]["out"], np.float64)))
        for c in range(N_CORES)
    )
    return np.float32((total - pad_corr) / M + LAM * reg)


# revision 87
# speedup vs baseline: 2.6191x; 1.1703x over previous
"""KGE (TransR-style) loss kernel for Trainium2, 8 NeuronCores.

Strategy:
  - Host: sort the M=8192 triples by relation id, pad each relation's
    segment to 64-row runs -> 80 blocks of 128 rows where every block is
    exactly two single-relation 64-row runs.  10 blocks/core, one SPMD
    program for all 8 cores.  W tables are packed one fp8 slot per run
    (duplicated as needed) so all device-side APs are static.  Each
    core's entity rows are deduplicated into a <=4096-row bf16 shard so
    gather ids fit int16 (dma_gather requirement).
  - Device (per core), software-pipelined over 5 gather chunks:
      * one dma_gather per chunk pulls the chunk's h/pos/neg rows from
        the entity shard into X = [H|P|N] per block (batched SWDGE: the
        994ns/instr descriptor-generation overhead dominated a
        per-block-indirect-DMA design)
      * fused subtract-transpose on PE: D_pos^T = H^T - P^T via two
        regular bf16 matmuls against [I | -I] accumulating in an f32
        PSUM bank (no separate DVE subtract stage), then one wide ACT
        copy per chunk to fp8 SBUF
      * per block pair (one PSUM bank): V = [D@W]+r via one K=32 one-hot
        bf16 matmul (start+stop, adds r to all four 128-col quadrants)
        then 8 fp8 64-row run matmuls accumulating with skip_group_check
      * ACT Square V -> bf16 squares; DVE computes neg_sq - pos_sq and a
        segmented reduce -> y = halved score diff per (row, block)
        (W and r are pre-scaled by sqrt(0.5) so no 0.5x is needed)
      * softplus(y) = relu(y) + ln1p(exp(-|y|)) with ln1p as a degree-2
        Horner polynomial on DVE (keeps ACT on one function table; a
        table switch costs 1283ns)
  - Final: each core DMAs its [128, NB] per-slot softplus values out;
    the host sums them, subtracts the exact pad-slot contribution
    n_pad*softplus(0), divides by M, and adds the 1e-5-scaled
    embedding-norm regularizer computed on host in fp64.
"""

import os
from contextlib import ExitStack

import numpy as np

import concourse.tile as tile
from concourse import bacc, mybir

M = 8192
E = 128
N_ENT = 500000
N_REL = 64
LAM = 1e-5
P = 128
RUN = 64
KSEL = 16
N_CORES = 8
UCAP = 4096  # per-core entity-shard capacity (unique rows <= 3840)
WARM_BRIDGE = 55  # PE warm-up matmuls bridging the pre-data idle window
WARM_GAP = 8  # PE warm-up matmuls per inter-chunk gap

f32 = mybir.dt.float32
bf16 = mybir.dt.bfloat16
f8 = mybir.dt.float8e4
i32 = mybir.dt.int32

# degree-2 near-minimax fit of ln(1+t) on [0,1], max err 6.3e-3 (the
# final loss error from this is ~3e-4 relative, tolerance is 2e-2)
LN1P_C = (0.006254230969353736, 0.9157555388644879, -0.23351351974861426)


def _softplus0_f32():
    """softplus(0) exactly as the device computes it (f32 ops, t=1)."""
    f = np.float32
    t = f(1.0)
    h1 = f(f(t * f(LN1P_C[2])) + f(LN1P_C[1]))
    h2 = f(h1 * t)
    h3 = f(h2 + f(LN1P_C[0]))
    return float(h3)


def _chunk_sizes(nq):
    if nq >= 5:
        return [1, 1, nq - 4, 1, 1]
    if nq >= 3:
        return [1] * nq
    return [nq]


_cache = {}


def _build(NB: int):
    """Build + compile the single-core SPMD program for NB blocks/core."""
    nc = bacc.Bacc(
        "TRN2",
        target_bir_lowering=False,
        debug=False,
        num_devices=N_CORES,
        # default 16KiB ring = 1024 SWDGE descriptors; the batched gathers
        # need ~4k descriptors in flight or they serialize on ring drain
        dynamic_dma_scratch_size=65536,
    )

    assert NB % 2 == 0, "block pairing assumes even NB"
    NQ = NB // 2  # block pairs; each pair shares one PSUM bank

    # per-core entity shard: the <=3840 rows this core's slots reference,
    # deduplicated and remapped to int16 ids (dma_gather takes int16)
    ent = nc.dram_tensor("entc", (UCAP, E), bf16, kind="ExternalInput").ap()
    # gather indices, 16-partition-wrapped per chunk (idx i of a chunk at
    # [i%16, chunk_col0 + i//16]), replicated over the 8 Q7 core groups
    idx3 = nc.dram_tensor("idx16", (P, 24 * NB), mybir.dt.int16, kind="ExternalInput").ap()
    wsl = nc.dram_tensor("wsl", (P, 2 * NB * P), f8, kind="ExternalInput").ap()
    # paired sel one-hot [2*KSEL, NQ*P] followed by r4 [2*KSEL, 4P]
    selr = nc.dram_tensor(
        "selr", (2 * KSEL, NQ * P + 4 * P), bf16, kind="ExternalInput"
    ).ap()
    idn = nc.dram_tensor("idn", (P, 2 * P), bf16, kind="ExternalInput").ap()
    # per-(row,block) softplus values; host does the final sum
    out = nc.dram_tensor("out", (P, NB), f32, kind="ExternalOutput").ap()

    # split the gather into chunks of block pairs for DMA/compute
    # pipelining; small first chunk starts compute early, small last
    # chunk keeps the post-last-gather chain short
    sizes_q = _chunk_sizes(NQ)
    chunks = []
    q0 = 0
    for cq in sizes_q:
        chunks.append((2 * q0, 2 * cq))
        q0 += cq

    with tile.TileContext(nc) as tc, ExitStack() as ctx:
        const = ctx.enter_context(tc.tile_pool(name="const", bufs=1))
        dtp = ctx.enter_context(tc.tile_pool(name="dtp", bufs=2, space="PSUM"))
        vp = ctx.enter_context(tc.tile_pool(name="vp", bufs=2, space="PSUM"))

        # ---- input loads ----
        # idx first: the gathers' descriptor generation waits on it
        # (hardware requires SBUF-resident gather indices)
        idx_sb = const.tile([P, 24 * NB], mybir.dt.int16)
        c0b = chunks[0][1] * 24  # chunk 0's index columns load first
        nc.sync.dma_start(out=idx_sb[:, 0:c0b], in_=idx3[:, 0:c0b])
        nc.sync.dma_start(out=idx_sb[:, c0b:], in_=idx3[:, c0b:])
        # W in two halves: the first (needed by the first quads) loads in
        # the pre-gather DMA window; the second is issued last so the DMA
        # engine FIFO runs it after the gathers instead of blocking them
        w_sb = const.tile([P, 2 * NB * P], f8)
        wh = NB * P  # half, in columns
        nc.sync.dma_start(out=w_sb[:, 0:wh], in_=wsl[:, 0:wh])
        selr_sb = const.tile([2 * KSEL, NQ * P + 4 * P], bf16)
        nc.sync.dma_start(out=selr_sb[:], in_=selr[:])
        idn_sb = const.tile([P, 2 * P], bf16)
        nc.sync.dma_start(out=idn_sb[:], in_=idn[:])
        nc.sync.dma_start(out=w_sb[:, wh:], in_=wsl[:, wh:])

        # PE p-state warm-up: the tensor engine only reaches full clock
        # after 3us of continuous execution and resets on idle.  Dependency-
        # free junk matmuls bridge PE's idle gaps so the real matmuls run
        # at full rate (2x the mid-p-state rate in the cost model).
        junk = const.tile([P, 2 * P], bf16)
        nc.gpsimd.memset(junk[:], 1.0)
        wp = ctx.enter_context(tc.tile_pool(name="wp", bufs=1, space="PSUM"))
        warm_ps = wp.tile([P, P], f32, name="warm_ps")

        def pe_warm(n):
            for _ in range(n):
                nc.tensor.matmul(
                    out=warm_ps[:], lhsT=junk[:, 0:P], rhs=junk[:, P : 2 * P],
                    start=True, stop=True,
                )

        xs = []
        for g, (b0, cg) in enumerate(chunks):
            x = const.tile([P, cg * 3 * E], bf16, tag=f"x{g}")
            n_idx = 3 * cg * P
            nc.gpsimd.dma_gather(
                x[:].rearrange("p (c e) -> p c e", e=E),
                ent[:],
                idx_sb[:, 24 * b0 : 24 * (b0 + cg)],
                n_idx,
                n_idx,
                E,
            )
            xs.append(x)

        # D_pos/D_neg per chunk: wide strided subtract, f32 -> bf16
        dT = const.tile([P, 2 * NB * E], f8)  # block b: pos at 2b, neg at 2b+1
        sq_all = const.tile([P, 2 * NB * E], bf16)
        dsq = const.tile([P, NB * E], bf16)
        # y[p, b] = sum(neg_sq - pos_sq); W,r pre-scaled by sqrt(0.5) so
        # this is already the halved score diff.  Pads give y=0 exactly;
        # the host subtracts n_pad*softplus(0).
        y = const.tile([P, NB], f32)

        r4 = selr_sb[:, NQ * P : NQ * P + 4 * P]

        def pass_a(g, copy_on_dve, fine=False):
            """fused subtract-transposes + D^T copies for chunk g.
            D_pos^T = H^T - P^T via two regular bf16 matmuls against
            [I | -I] accumulating in PSUM (replaces DVE subtract + PE
            transpose-mode: one less relay stage per chunk)."""
            b0, cg = chunks[g]
            x3 = xs[g][:].rearrange("p (b x) -> p b x", x=3 * E)
            for g4 in range(0, cg, 4):
                n4 = min(4, cg - g4)
                dt_ps = dtp.tile([P, 8 * E], f32, tag="dt")
                for i in range(n4):
                    bl = g4 + i  # block local to chunk
                    xh = x3[:, bl, 0:E]
                    for s, xt in enumerate((x3[:, bl, E : 2 * E],
                                            x3[:, bl, 2 * E : 3 * E])):
                        o = dt_ps[:, (2 * i + s) * E : (2 * i + s + 1) * E]
                        nc.tensor.matmul(
                            out=o, lhsT=xh, rhs=idn_sb[:, 0:P],
                            start=True, stop=False,
                        )
                        nc.tensor.matmul(
                            out=o, lhsT=xt, rhs=idn_sb[:, P : 2 * P],
                            start=False, stop=True,
                        )
                dst0 = 2 * (b0 + g4) * E
                if copy_on_dve:
                    # DVE copy of bf16 PSUM runs at 2x and fills a DVE gap
                    nc.vector.tensor_scalar_add(
                        out=dT[:, dst0 : dst0 + 2 * n4 * E],
                        in0=dt_ps[:, 0 : 2 * n4 * E],
                        scalar1=0.0,
                    )
                elif fine:
                    # per-block copies: downstream matmuls start sooner on
                    # the tail chunk
                    for i in range(n4):
                        nc.scalar.copy(
                            dT[:, dst0 + 2 * i * E : dst0 + 2 * (i + 1) * E],
                            dt_ps[:, 2 * i * E : 2 * (i + 1) * E],
                        )
                else:
                    nc.scalar.copy(
                        dT[:, dst0 : dst0 + 2 * n4 * E], dt_ps[:, 0 : 2 * n4 * E]
                    )

        def _sq_red(qg, nq2, v_ps, off, nblk, dve_square=False):
            """square + diff + segmented reduce over nblk blocks -> y cols."""
            bg = 2 * qg + (off // 256)
            if dve_square:
                # DVE square via same-tile PSUM double-read mult: relieves
                # the saturated ACT engine for the tail chunks
                nc.vector.tensor_tensor(
                    out=sq_all[:, bg * 256 : (bg + nblk) * 256],
                    in0=v_ps[:, off : off + nblk * 256],
                    in1=v_ps[:, off : off + nblk * 256],
                    op=mybir.AluOpType.mult,
                )
            else:
                nc.scalar.activation(
                    out=sq_all[:, bg * 256 : (bg + nblk) * 256],
                    in_=v_ps[:, off : off + nblk * 256],
                    func=mybir.ActivationFunctionType.Square,
                )
            sq4 = sq_all[:, bg * 256 : (bg + nblk) * 256].rearrange(
                "p (b two x) -> p b two x", two=2, x=E
            )
            nc.vector.tensor_tensor(
                out=dsq[:, bg * E : (bg + nblk) * E].rearrange(
                    "p (b x) -> p b x", x=E
                ),
                in0=sq4[:, :, 1:2, :].squeeze(2),
                in1=sq4[:, :, 0:1, :].squeeze(2),
                op=mybir.AluOpType.subtract,
            )
            nc.vector.reduce_sum(
                out=y[:, bg : bg + nblk],
                in_=dsq[:, bg * E : (bg + nblk) * E].rearrange(
                    "p (b x) -> p b x", x=E
                ),
                axis=mybir.AxisListType.X,
            )

        def pass_b1(g, fine=False, dve_square=False):
            """matmuls per pair + square/diff/reduce per quad (or per block
            when fine, so the tail chunk's serial chain is shortest)."""
            b0, cg = chunks[g]
            q0g = b0 // 2
            nqg = cg // 2
            for qa in range(0, nqg, 2):
                nq2 = min(2, nqg - qa)
                v_ps = vp.tile([P, nq2 * 512], f32, tag="v")
                for ql in range(qa, qa + nq2):
                    q = q0g + ql
                    off = (ql - qa) * 512
                    # r add for both blocks of the pair: K=32 one-hot
                    # matmul.  start+stop here; the partition-sliced run
                    # matmuls accumulate with skip_group_check (the sim's
                    # zero-region bookkeeping mishandles partition-offset
                    # outputs; the accumulation itself is exact)
                    nc.tensor.matmul(
                        out=v_ps[:, off : off + 512],
                        lhsT=selr_sb[:, q * P : (q + 1) * P],
                        rhs=r4,
                        start=True,
                        stop=True,
                    )
                    for j, b in enumerate((2 * q, 2 * q + 1)):
                        dposT = dT[:, (2 * b) * E : (2 * b + 1) * E]
                        dnegT = dT[:, (2 * b + 1) * E : (2 * b + 2) * E]
                        c4 = off + 256 * j
                        for u in range(2):
                            wslot = w_sb[:, (2 * b + u) * P : (2 * b + u + 1) * P]
                            rows = slice(u * RUN, (u + 1) * RUN)
                            nc.tensor.matmul(
                                out=v_ps[rows, c4 : c4 + 128],
                                lhsT=dposT[:, rows],
                                rhs=wslot,
                                start=False,
                                stop=False,
                                skip_group_check=True,
                            )
                            nc.tensor.matmul(
                                out=v_ps[rows, c4 + 128 : c4 + 256],
                                lhsT=dnegT[:, rows],
                                rhs=wslot,
                                start=False,
                                stop=False,
                                skip_group_check=True,
                            )
                        if fine:
                            _sq_red(q0g + qa, nq2, v_ps, c4, 1)
                if not fine:
                    _sq_red(q0g + qa, nq2, v_ps, 0, 2 * nq2, dve_square)

        # software-pipelined emission: A_g one chunk ahead of B_{g-1};
        # the last chunk runs at per-block granularity for a short tail
        ng = len(chunks)
        pe_warm(WARM_BRIDGE)
        pass_a(0, copy_on_dve=False)
        for g in range(1, ng):
            pass_a(g, copy_on_dve=False)
            pass_b1(g - 1, dve_square=(g - 1 in DVE_SQ))
            pe_warm(WARM_GAP)
        pass_b1(ng - 1, dve_square=(ng - 1 in DVE_SQ))

        # ---- batched tail ----
        # softplus(y) = relu(y) + ln(1 + t), t = exp(-|y|).  ln(1+t) is a
        # degree-4 polynomial on DVE (max err 1.4e-4 on [0,1]) so the ACT
        # engine never needs the Ln table (saves a 1283ns table reload)
        t_abs = const.tile([P, NB], f32)
        nc.scalar.activation(
            out=t_abs[:], in_=y[:], func=mybir.ActivationFunctionType.Abs
        )
        t = const.tile([P, NB], f32)
        nc.scalar.activation(
            out=t[:], in_=t_abs[:], func=mybir.ActivationFunctionType.Exp,
            scale=-1.0,
        )
        # Horner, depth 3: ln1p(t) ~ (C2*t + C1)*t + C0
        h1 = const.tile([P, NB], f32)
        nc.vector.tensor_scalar(
            out=h1[:], in0=t[:], scalar1=LN1P_C[2], scalar2=LN1P_C[1],
            op0=mybir.AluOpType.mult, op1=mybir.AluOpType.add,
        )
        nc.vector.tensor_tensor(
            out=h1[:], in0=h1[:], in1=t[:], op=mybir.AluOpType.mult
        )
        t_relu = const.tile([P, NB], f32)
        nc.scalar.activation(
            out=t_relu[:], in_=y[:], func=mybir.ActivationFunctionType.Relu,
        )
        nc.vector.tensor_scalar_add(out=h1[:], in0=h1[:], scalar1=LN1P_C[0])
        acc = const.tile([P, NB], f32)
        nc.vector.tensor_tensor(
            out=acc[:], in0=h1[:], in1=t_relu[:], op=mybir.AluOpType.add
        )
        nc.sync.dma_start(out=out[:], in_=acc[:])

    nc.compile()
    return nc


def _plan(h, r, pos_t, neg_t, relation_weight, relation_embed):
    """Sort by relation, pad to 64-row single-relation runs, split 8 ways."""
    order = np.argsort(r, kind="stable")
    counts = np.bincount(r, minlength=N_REL)
    rows_parts, rel_parts = [], []
    pos = 0
    for k in range(N_REL):
        c = int(counts[k])
        ids = order[pos : pos + c]
        pos += c
        if c == 0:
            continue
        n_slots = -(-c // RUN) * RUN
        arr = np.full(n_slots, -1, np.int64)
        arr[:c] = ids
        rows_parts.append(arr)
        rel_parts.append(np.full(n_slots, k, np.int64))
    rows = np.concatenate(rows_parts)
    rels = np.concatenate(rel_parts)
    t0 = len(rows)
    nb = max(2, -(-t0 // (P * N_CORES)))
    nb += nb % 2  # block pairing requires even nb
    t = nb * P * N_CORES
    if t > t0:
        rows = np.concatenate([rows, np.full(t - t0, -1, np.int64)])
        rels = np.concatenate([rels, np.zeros(t - t0, np.int64)])

    eye = np.eye(P, dtype=np.float32)
    iden = np.concatenate([eye, -eye], axis=1)  # [I | -I] for the fused
    # subtract-transpose matmuls

    maps = []
    for c in range(N_CORES):
        sl_rows = rows[c * nb * P : (c + 1) * nb * P]
        sl_rels = rels[c * nb * P : (c + 1) * nb * P]
        core_rels = []
        for k in sl_rels[::RUN]:
            if k not in core_rels:
                core_rels.append(int(k))
        assert len(core_rels) <= KSEL, f"core {c} has {len(core_rels)} relations"
        rel_slot = {k: i for i, k in enumerate(core_rels)}

        nq = nb // 2
        idx3 = np.zeros((P, nb, 3), np.int32)
        wv = np.zeros((P, 2 * nb, P), np.float32)
        sel = np.zeros((2 * KSEL, nq * P + 4 * P), np.float32)
        s5 = np.float32(np.sqrt(0.5))
        for b in range(nb):
            for u in range(2):
                k = int(sl_rels[(2 * b + u) * RUN])
                wv[:, 2 * b + u, :] = relation_weight[k] * s5
            q, j = divmod(b, 2)
            for p in range(P):
                s = b * P + p
                rid = sl_rows[s]
                if rid >= 0:
                    idx3[p, b, 0] = h[rid]
                    idx3[p, b, 1] = pos_t[rid]
                    idx3[p, b, 2] = neg_t[rid]
                    sel[j * KSEL + rel_slot[int(sl_rels[s])], q * P + p] = 1.0
        # r4 = [[r|r], 0; 0, [r|r]] so one K=32 matmul adds r to both
        # blocks of a pair (pos and neg halves)
        for i, k in enumerate(core_rels):
            rrow = relation_embed[k] * s5
            for rep in range(2):
                sel[i, nq * P + rep * P : nq * P + (rep + 1) * P] = rrow
                sel[KSEL + i, nq * P + (2 + rep) * P : nq * P + (3 + rep) * P] = rrow
        # dedup entity rows for this core; remap slot ids to int16 into the
        # compact shard.  Wrap each gather chunk's index list 16-wide (idx i
        # at [i%16, i//16]) and replicate across the 8 Q7 core groups.
        uids, inv = np.unique(idx3.reshape(-1), return_inverse=True)
        assert len(uids) <= UCAP, f"core {c}: {len(uids)} unique rows"
        cid = inv.reshape(P, nb, 3).astype(np.int16)
        sizes_q = _chunk_sizes(nb // 2)
        idx16 = np.zeros((P, 24 * nb), np.int16)
        b0 = 0
        for cq in sizes_q:
            cg = 2 * cq
            # L[i] = cid[p=i%128, col=3*b0 + i//128] for the chunk's slots
            lcol = cid[:, b0 : b0 + cg, :].reshape(P, 3 * cg, order="C")
            L = lcol.T.reshape(-1)  # col-major: i = c*128 + p
            wrapped = L.reshape(-1, 16).T  # [16, 24*cg]
            idx16[:, 24 * b0 : 24 * (b0 + cg)] = np.tile(wrapped, (8, 1))
            b0 += cg
        maps.append(
            {
                "uids": uids,
                "idx16": idx16,
                "wsl": wv.reshape(P, 2 * nb * P),
                "selr": sel,
                "idn": iden,
            }
        )
    n_pad = len(rows) - M
    return nb, n_pad, maps


def _to_bf16(a):
    import ml_dtypes

    return np.asarray(a, dtype=np.float32).astype(ml_dtypes.bfloat16)


def _to_f8(a):
    import ml_dtypes

    return np.asarray(a, dtype=np.float32).astype(ml_dtypes.float8_e4m3)


def kernel(h, r, pos_t, neg_t, entity_embed, relation_embed, relation_weight):
    h = np.asarray(h).astype(np.int64)
    r = np.asarray(r).astype(np.int64)
    pos_t = np.asarray(pos_t).astype(np.int64)
    neg_t = np.asarray(neg_t).astype(np.int64)
    import ml_dtypes

    ent_f32 = np.ascontiguousarray(np.asarray(entity_embed, dtype=np.float32))
    ent = np.ascontiguousarray(ent_f32.astype(ml_dtypes.bfloat16))
    re_ = np.ascontiguousarray(np.asarray(relation_embed, dtype=np.float32))
    rw = np.ascontiguousarray(np.asarray(relation_weight, dtype=np.float32))

    nb, n_pad, maps = _plan(h, r, pos_t, neg_t, rw, re_)
    if nb not in _cache:
        _cache[nb] = _build(nb)
    nc = _cache[nb]

    in_maps = []
    for c in range(N_CORES):
        m = maps[c]
        entc = np.zeros((UCAP, E), ent.dtype)
        entc[: len(m["uids"])] = ent[m["uids"]]
        in_maps.append(
            {
                "entc": entc,
                "idx16": m["idx16"],
                "wsl": _to_f8(m["wsl"]),
                "selr": _to_bf16(m["selr"]),
                "idn": _to_bf16(m["idn"]),
            }
        )

    # host-side regularizer (1e-5-scaled, ~1e-4 of the loss; fp64 exact)
    he = ent_f32[h].astype(np.float64)
    pe = ent_f32[pos_t].astype(np.float64)
    ne = ent_f32[neg_t].astype(np.float64)
    rr = re_[r].astype(np.float64)
    reg = (
        np.mean(np.sum(he * he, axis=1)) / 2.0
        + np.mean(np.sum(rr * rr, axis=1)) / 2.0
        + np.mean(np.sum(pe * pe, axis=1)) / 2.0
        + np.mean(np.sum(ne * ne, axis=1)) / 2.0
    )

    pad_corr = n_pad * _softplus0_f32()

    if os.environ.get("KGE_SIM"):
        from concourse.bass_interp import CoreSim

        total = 0.0
        ncores = int(os.environ.get("KGE_SIM_CORES", N_CORES))
        for c in range(ncores):
            sim = CoreSim(nc, trace=False)
            for name, arr in in_maps[c].items():
                sim.tensor(name)[:] = arr
            sim.simulate()
            total += float(np.sum(np.asarray(sim.tensor("out"), np.float64)))
        if ncores < N_CORES:
            return np.float32(total)  # partial, dev only
        return np.float32((total - pad_corr) / M + LAM * reg)

    from concourse.bass_utils import run_bass_kernel_spmd

    res = run_bass_kernel_spmd(nc, in_maps, core_ids=list(range(N_CORES)))
    total = sum(
        float(np.sum(np.asarray(res.results[c]["out"], np.float64)))
        for c in range(N_CORES)
    )
    return np.float32((total - pad_corr) / M + LAM * reg)
